# revision 4
# baseline (speedup 1.0000x reference)
"""Bass/Trainium2 kernel for nn_DifferentSoftQNetwork.

Math: the reference is three chained per-sample-expert matmuls with NO
nonlinearity:
    out[b] = state[b] @ W1[o_b] @ W2[o_b] @ W3[o_b],   o_b = option[b]
Because it is linear, collapse the weight chain per expert:
    v[e] = W1[e] @ W2[e] @ W3[e]  in R^128
    out[b] = dot(state[b], v[o_b])
This turns a 672-MFLOP batched matmul into a memory-bound stream of the
weights (~21 MB).

Sharding: experts are sharded across the 8 cores (2 experts per core);
state/option are replicated.  Each core computes scores s[e, b] for its two
experts and masks them by (option == e); the host sums the 8 partial
[2, 1024] outputs (each b matches exactly one (core, expert) pair).

Per-core device program (experts e0, e1):
  ST        = state^T                      (PE transpose via identity)
  W1T[e]    = W1[e]^T                      (PE transpose)
  CT[e]     = W1T[e]^T @ W2[e] = W1[e] @ W2[e]   ([128i, 512k] in PSUM)
  V[:, e]   = reduce_k(CT[e] * bcast(W3[e]))     (DVE fused mul+reduce)
  sT        = V^T @ ST                           ([2, 1024] scores)
  out[e, b] = sT[e, b] * (option[b] == e)        (DVE is_equal + mul)
"""

import numpy as np

B, I, H, O = 1024, 128, 512, 16
NCORES = 8
EPC = O // NCORES  # experts per core = 2

_CACHE = {}

# Internal knobs for the local test harness (the grading harness never
# touches these): when _TRACE is set, the next kernel() call runs with
# NTFF profiling and stores the BassKernelResults in _LAST_RESULTS.
_TRACE = False
_LAST_RESULTS = None


def _build_nc():
    import concourse.bacc as bacc
    import concourse.bass as bass
    import concourse.mybir as mybir
    import concourse.tile as tile

    fp32 = mybir.dt.float32
    P = 128

    nc = bacc.Bacc("TRN2", target_bir_lowering=False, debug=False)

    state_d = nc.dram_tensor("state", [B, I], fp32, kind="ExternalInput")
    w1_d = nc.dram_tensor("w1", [EPC, I, H], fp32, kind="ExternalInput")
    w2_d = nc.dram_tensor("w2", [EPC, H, H], fp32, kind="ExternalInput")
    w3_d = nc.dram_tensor("w3", [EPC, H], fp32, kind="ExternalInput")
    optf_d = nc.dram_tensor("optf", [1, B], fp32, kind="ExternalInput")
    ce_d = nc.dram_tensor("ce", [EPC, 1], fp32, kind="ExternalInput")
    ident_d = nc.dram_tensor("ident", [P, P], fp32, kind="ExternalInput")
    out_d = nc.dram_tensor("outp", [EPC, B], fp32, kind="ExternalOutput")

    NB = B // P  # 8 state tiles
    NH = H // P  # 4 chunks of the hidden dim

    with tile.TileContext(nc) as tc:
        with (
            tc.tile_pool(name="sb", bufs=1) as sb,
            tc.tile_pool(name="sb2", bufs=2) as sb2,
            tc.tile_pool(name="pst", bufs=2, space=bass.MemorySpace.PSUM) as pst,
            tc.tile_pool(name="psc", bufs=2, space=bass.MemorySpace.PSUM) as psc,
            tc.tile_pool(name="pss", bufs=2, space=bass.MemorySpace.PSUM) as pss,
        ):
            ident = sb.tile([P, P], fp32, tag="ident", name="ident")
            nc.sync.dma_start(ident[:], ident_d[:])

            optf = sb.tile([EPC, B], fp32, tag="optf", name="optf")
            nc.sync.dma_start(optf[:], optf_d[0:1, :].to_broadcast([EPC, B]))

            ce = sb.tile([EPC, 1], fp32, tag="ce", name="ce")
            nc.sync.dma_start(ce[:], ce_d[:])

            # state^T: 8 PE transposes of [128, 128] tiles into one wide tile
            ST = sb.tile([P, B], fp32, tag="ST", name="ST")
            for t in range(NB):
                s = sb.tile([P, I], fp32, tag=f"s{t}", name=f"s{t}")
                nc.sync.dma_start(s[:], state_d[t * P : (t + 1) * P, :])
                pt = pst.tile([P, P], fp32, tag="pt", name="pt")
                nc.tensor.transpose(pt[:], s[:], ident[:])
                nc.vector.tensor_copy(ST[:, t * P : (t + 1) * P], pt[:])

            # W1[e]^T chunks
            w1t = {}
            for e in range(EPC):
                w1s = sb.tile([I, H], fp32, tag=f"w1_{e}", name=f"w1_{e}")
                nc.sync.dma_start(w1s[:], w1_d[e])
                for c in range(NH):
                    pw = pst.tile([P, P], fp32, tag="pt", name="pt")
                    nc.tensor.transpose(pw[:], w1s[:, c * P : (c + 1) * P], ident[:])
                    w1t[e, c] = sb.tile([P, P], fp32, tag=f"w1t_{e}_{c}", name=f"w1t_{e}_{c}")
                    nc.vector.tensor_copy(w1t[e, c][:], pw[:])

            # V[:, e] = reduce_k( (W1[e] @ W2[e]) * bcast(W3[e]) )
            V = sb.tile([P, EPC], fp32, tag="V", name="V")
            for e in range(EPC):
                ct = psc.tile([P, H], fp32, tag="ct", name="ct")
                for h in range(NH):
                    w2s = sb.tile([P, H], fp32, tag=f"w2_{e}_{h}", name=f"w2_{e}_{h}")
                    nc.sync.dma_start(w2s[:], w2_d[e, h * P : (h + 1) * P, :])
                    nc.tensor.matmul(
                        ct[:], w1t[e, h][:], w2s[:], start=(h == 0), stop=(h == NH - 1)
                    )
                w3b = sb.tile([P, H], fp32, tag=f"w3b_{e}", name=f"w3b_{e}")
                nc.sync.dma_start(w3b[:], w3_d[e : e + 1, :].to_broadcast([P, H]))
                junk = sb2.tile([P, H], fp32, tag="junk", name="junk")
                nc.vector.tensor_mul(junk[:], ct[:], w3b[:])
                nc.vector.tensor_reduce(
                    V[:, e : e + 1],
                    junk[:],
                    axis=mybir.AxisListType.X,
                    op=mybir.AluOpType.add,
                )

            # scores sT = V^T @ ST, then mask by (option == e)
            outp = sb.tile([EPC, B], fp32, tag="outp", name="outp")
            NS = 512
            for half in range(B // NS):
                stp = pss.tile([EPC, NS], fp32, tag="stp", name="stp")
                nc.tensor.matmul(
                    stp[:],
                    V[:],
                    ST[:, half * NS : (half + 1) * NS],
                    start=True,
                    stop=True,
                )
                eq = sb2.tile([EPC, NS], fp32, tag="eq", name="eq")
                nc.vector.tensor_scalar(
                    eq[:],
                    optf[:, half * NS : (half + 1) * NS],
                    ce[:],
                    None,
                    op0=mybir.AluOpType.is_equal,
                )
                nc.vector.tensor_mul(
                    outp[:, half * NS : (half + 1) * NS], stp[:], eq[:]
                )
            nc.sync.dma_start(out_d[:], outp[:])

    nc.compile()
    return nc


def _get_nc():
    if "nc" not in _CACHE:
        _CACHE["nc"] = _build_nc()
    return _CACHE["nc"]


def kernel(state, action, W1, W2, W3, option):
    global _LAST_RESULTS
    from concourse import bass_utils

    nc = _get_nc()

    state = np.ascontiguousarray(np.asarray(state), dtype=np.float32)
    W1 = np.asarray(W1, dtype=np.float32)
    W2 = np.asarray(W2, dtype=np.float32)
    W3 = np.asarray(W3, dtype=np.float32)
    optf = np.asarray(option).astype(np.float32).reshape(1, B)
    ident = np.eye(128, dtype=np.float32)

    in_maps = []
    for c in range(NCORES):
        e0 = EPC * c
        in_maps.append(
            {
                "state": state,
                "w1": np.ascontiguousarray(W1[e0 : e0 + EPC]),
                "w2": np.ascontiguousarray(W2[e0 : e0 + EPC]),
                "w3": np.ascontiguousarray(W3[e0 : e0 + EPC, :, 0]),
                "optf": optf,
                "ce": np.arange(e0, e0 + EPC, dtype=np.float32).reshape(EPC, 1),
                "ident": ident,
            }
        )

    res = bass_utils.run_bass_kernel_spmd(
        nc, in_maps, core_ids=list(range(NCORES)), trace=_TRACE
    )
    _LAST_RESULTS = res

    out = np.zeros((B,), np.float32)
    for c in range(NCORES):
        out += res.results[c]["outp"].sum(axis=0)
    return out.reshape(B, 1)


# revision 7
# speedup vs baseline: 1.0783x; 1.0783x over previous
"""Bass/Trainium2 kernel for nn_DifferentSoftQNetwork.

Math: the reference is three chained per-sample-expert matmuls with NO
nonlinearity:
    out[b] = state[b] @ W1[o_b] @ W2[o_b] @ W3[o_b],   o_b = option[b]
Because it is linear, collapse the weight chain per expert:
    v[e] = W1[e] @ W2[e] @ W3[e]  in R^128
    out[b] = dot(state[b], v[o_b])
This turns a 672-MFLOP batched matmul into a memory-bound stream of the
weights (~21 MB).

Sharding: experts are sharded across the 8 cores (2 experts per core);
state/option are replicated.  Each core computes scores s[e, b] for its two
experts and masks them by (option == e); the host sums the 8 partial
[2, 1024] outputs (each b matches exactly one (core, expert) pair).

Input marshalling on the host passes state and W1 pre-transposed (stateT
[I, B], W1T [E, H, I]) so every device-side matmul consumes its natural
layout directly (PE matmul computes lhsT.T @ rhs and needs the contraction
dim on partitions for both operands).

Per-core device program (experts e0, e1):
  CT[e]     = W1T[e].T @ W2[e] = W1[e] @ W2[e]    ([128i, 512k] in PSUM)
  w3b[e]    = partition_broadcast(W3[e])          (GpSimd)
  V[:, e]   = reduce_k(CT[e] * w3b[e])            (DVE mul + reduce)
  sT        = V^T @ ST                            ([2, 1024] scores)
  out[e, b] = sT[e, b] * (option[b] == e)         (DVE is_equal + mul)

DMA issue is split across the two HWDGE queues (sync + scalar) with few,
large transfers; the host-side gather sums the partial outputs.
"""

import numpy as np

B, I, H, O = 1024, 128, 512, 16
NCORES = 8
EPC = O // NCORES  # experts per core = 2

_CACHE = {}

# Internal knobs for the local test harness (the grading harness never
# touches these): when _TRACE is set, the next kernel() call runs with
# NTFF profiling and stores the BassKernelResults in _LAST_RESULTS.
_TRACE = False
_LAST_RESULTS = None


def _build_nc():
    import concourse.bacc as bacc
    import concourse.bass as bass
    import concourse.mybir as mybir
    import concourse.tile as tile

    fp32 = mybir.dt.float32
    P = 128

    nc = bacc.Bacc("TRN2", target_bir_lowering=False, debug=False)

    statet_d = nc.dram_tensor("statet", [I, B], fp32, kind="ExternalInput")
    w1t_d = nc.dram_tensor("w1t", [EPC, H, I], fp32, kind="ExternalInput")
    w2_d = nc.dram_tensor("w2", [EPC, H, H], fp32, kind="ExternalInput")
    w3_d = nc.dram_tensor("w3", [EPC, H], fp32, kind="ExternalInput")
    optf_d = nc.dram_tensor("optf", [1, B], fp32, kind="ExternalInput")
    ce_d = nc.dram_tensor("ce", [EPC, 1], fp32, kind="ExternalInput")
    out_d = nc.dram_tensor("outp", [EPC, B], fp32, kind="ExternalOutput")

    NH = H // P  # 4 chunks of the hidden dim
    NS = 512  # max moving free dim per matmul

    with tile.TileContext(nc) as tc:
        with (
            tc.tile_pool(name="sb", bufs=1) as sb,
            tc.tile_pool(name="sb2", bufs=2) as sb2,
            tc.tile_pool(name="psc", bufs=2, space=bass.MemorySpace.PSUM) as psc,
            tc.tile_pool(name="pss", bufs=2, space=bass.MemorySpace.PSUM) as pss,
        ):
            # ---- DMA loads: few and large, split across the two HWDGE
            # queues (sync, scalar), ordered by first use.
            # w1t[e]: [128, 512] tile; column block c holds W1[e].T chunk c
            # (partition = h within chunk).  DRAM view [H, I] -> [c, p, i].
            w1t_view = w1t_d.rearrange("e (c p) i -> e p c i", p=P)
            w1ts = []
            for e in range(EPC):
                t = sb.tile([P, NH * I], fp32, tag=f"w1t_{e}", name=f"w1t_{e}")
                eng = nc.sync if e == 0 else nc.scalar
                eng.dma_start(t[:], w1t_view[e])
                w1ts.append(t)

            # w2[e]: [128, 2048] tile; column block h holds W2[e] rows
            # h*128..h*128+127 (partition = h within chunk).
            w2_view = w2_d.rearrange("e (h p) k -> e p h k", p=P)
            w2s = []
            for e in range(EPC):
                t = sb.tile([P, NH * H], fp32, tag=f"w2_{e}", name=f"w2_{e}")
                eng = nc.sync if e == 0 else nc.scalar
                eng.dma_start(t[:], w2_view[e])
                w2s.append(t)

            # state^T as one contiguous [128, 1024] load (host-transposed).
            ST = sb.tile([I, B], fp32, tag="ST", name="ST")
            nc.sync.dma_start(ST[:], statet_d[:])

            # small tensors on the scalar queue
            w3rs = []
            for e in range(EPC):
                t = sb.tile([1, H], fp32, tag=f"w3r_{e}", name=f"w3r_{e}")
                nc.scalar.dma_start(t[:], w3_d[e : e + 1, :])
                w3rs.append(t)
            optf = sb.tile([EPC, B], fp32, tag="optf", name="optf")
            nc.scalar.dma_start(optf[:], optf_d[0:1, :].to_broadcast([EPC, B]))
            ce = sb.tile([EPC, 1], fp32, tag="ce", name="ce")
            nc.scalar.dma_start(ce[:], ce_d[:])

            # ---- GpSimd: broadcast W3[e] rows across partitions
            w3bs = []
            for e in range(EPC):
                t = sb.tile([P, H], fp32, tag=f"w3b_{e}", name=f"w3b_{e}")
                nc.gpsimd.partition_broadcast(t[:], w3rs[e][:])
                w3bs.append(t)

            # ---- selection masks (early; only needs optf + ce)
            eqs = []
            for half in range(B // NS):
                eq = sb2.tile([EPC, NS], fp32, tag="eq", name="eq")
                nc.vector.tensor_scalar(
                    eq[:],
                    optf[:, half * NS : (half + 1) * NS],
                    ce[:],
                    None,
                    op0=mybir.AluOpType.is_equal,
                )
                eqs.append(eq)

            # ---- CT[e] = W1[e] @ W2[e], then V[:, e] = CT[e] @ W3[e]
            V = sb.tile([P, EPC], fp32, tag="V", name="V")
            for e in range(EPC):
                ct = psc.tile([P, H], fp32, tag="ct", name="ct")
                for h in range(NH):
                    nc.tensor.matmul(
                        ct[:],
                        w1ts[e][:, h * I : (h + 1) * I],
                        w2s[e][:, h * H : (h + 1) * H],
                        start=(h == 0),
                        stop=(h == NH - 1),
                    )
                junk = sb2.tile([P, H], fp32, tag="junk", name="junk")
                nc.vector.tensor_mul(junk[:], ct[:], w3bs[e][:])
                nc.vector.tensor_reduce(
                    V[:, e : e + 1],
                    junk[:],
                    axis=mybir.AxisListType.X,
                    op=mybir.AluOpType.add,
                )

            # ---- scores and masked output
            outp = sb.tile([EPC, B], fp32, tag="outp", name="outp")
            for half in range(B // NS):
                stp = pss.tile([EPC, NS], fp32, tag="stp", name="stp")
                nc.tensor.matmul(
                    stp[:],
                    V[:],
                    ST[:, half * NS : (half + 1) * NS],
                    start=True,
                    stop=True,
                )
                nc.vector.tensor_mul(
                    outp[:, half * NS : (half + 1) * NS], stp[:], eqs[half][:]
                )
            nc.sync.dma_start(out_d[:], outp[:])

    nc.compile()
    return nc


def _get_nc():
    if "nc" not in _CACHE:
        _CACHE["nc"] = _build_nc()
    return _CACHE["nc"]


def kernel(state, action, W1, W2, W3, option):
    global _LAST_RESULTS
    from concourse import bass_utils

    nc = _get_nc()

    state = np.asarray(state, dtype=np.float32)
    statet = np.ascontiguousarray(state.T)
    W1 = np.asarray(W1, dtype=np.float32)
    w1t = np.ascontiguousarray(np.transpose(W1, (0, 2, 1)))  # [O, H, I]
    W2 = np.asarray(W2, dtype=np.float32)
    W3 = np.asarray(W3, dtype=np.float32)
    optf = np.asarray(option).astype(np.float32).reshape(1, B)

    in_maps = []
    for c in range(NCORES):
        e0 = EPC * c
        in_maps.append(
            {
                "statet": statet,
                "w1t": np.ascontiguousarray(w1t[e0 : e0 + EPC]),
                "w2": np.ascontiguousarray(W2[e0 : e0 + EPC]),
                "w3": np.ascontiguousarray(W3[e0 : e0 + EPC, :, 0]),
                "optf": optf,
                "ce": np.arange(e0, e0 + EPC, dtype=np.float32).reshape(EPC, 1),
            }
        )

    res = bass_utils.run_bass_kernel_spmd(
        nc, in_maps, core_ids=list(range(NCORES)), trace=_TRACE
    )
    _LAST_RESULTS = res

    out = np.zeros((B,), np.float32)
    for c in range(NCORES):
        out += res.results[c]["outp"].sum(axis=0)
    return out.reshape(B, 1)


# revision 14
# speedup vs baseline: 1.1106x; 1.0300x over previous
"""Bass/Trainium2 kernel for nn_DifferentSoftQNetwork.

Math: the reference is three chained per-sample-expert matmuls with NO
nonlinearity:
    out[b] = state[b] @ W1[o_b] @ W2[o_b] @ W3[o_b],   o_b = option[b]
Because it is linear, collapse the weight chain per expert:
    v[e] = W1[e] @ W2[e] @ W3[e]  in R^128
    out[b] = dot(state[b], v[o_b])
This turns a 672-MFLOP batched matmul into a memory-bound stream of the
weights (~21 MB).

Sharding: experts are sharded across the 8 cores (2 experts per core);
state/option are replicated.  Each core computes scores s[e, b] for its two
experts and masks them by (option == e); the host sums the 8 partial
[2, 1024] outputs (each b matches exactly one (core, expert) pair).

Input marshalling on the host passes state and W1 pre-transposed (stateT
[I, B], W1T [E, H, I]) so every device-side matmul consumes its natural
layout directly (PE matmul computes lhsT.T @ rhs and needs the contraction
dim on partitions for both operands).

Per-core device program (experts e0, e1):
  CT[e]     = W1T[e].T @ W2[e] = W1[e] @ W2[e]    ([128i, 512k] in PSUM)
  w3b[e]    = partition_broadcast(W3[e])          (GpSimd)
  V[:, e]   = reduce_k(CT[e] * w3b[e])            (DVE mul + reduce)
  sT        = V^T @ ST                            ([2, 1024] scores)
  out[e, b] = sT[e, b] * (option[b] == e)         (DVE is_equal + mul)

DMA issue is split across the two HWDGE queues (sync + scalar) with few,
large transfers; the host-side gather sums the partial outputs.
"""

import numpy as np

B, I, H, O = 1024, 128, 512, 16
NCORES = 8
EPC = O // NCORES  # experts per core = 2

_CACHE = {}

# Internal knobs for the local test harness (the grading harness never
# touches these): when _TRACE is set, the next kernel() call runs with
# NTFF profiling and stores the BassKernelResults in _LAST_RESULTS.
_TRACE = False
_LAST_RESULTS = None


def _build_nc():
    import concourse.bacc as bacc
    import concourse.bass as bass
    import concourse.mybir as mybir
    import concourse.tile as tile

    fp32 = mybir.dt.float32
    P = 128

    nc = bacc.Bacc("TRN2", target_bir_lowering=False, debug=False)

    statet_d = nc.dram_tensor("statet", [I, B], fp32, kind="ExternalInput")
    w1t_d = nc.dram_tensor("w1t", [EPC, H, I], fp32, kind="ExternalInput")
    w2_d = nc.dram_tensor("w2", [EPC, H, H], fp32, kind="ExternalInput")
    w3_d = nc.dram_tensor("w3", [EPC, H], fp32, kind="ExternalInput")
    optf_d = nc.dram_tensor("optf", [1, B], fp32, kind="ExternalInput")
    out_d = nc.dram_tensor("outp", [EPC, B], fp32, kind="ExternalOutput")

    NH = H // P  # 4 chunks of the hidden dim
    NS = 512  # max moving free dim per matmul

    with tile.TileContext(nc) as tc:
        with (
            tc.tile_pool(name="sb", bufs=1) as sb,
            tc.tile_pool(name="sb2", bufs=2) as sb2,
            tc.tile_pool(name="psc", bufs=2, space=bass.MemorySpace.PSUM) as psc,
            tc.tile_pool(name="pss", bufs=2, space=bass.MemorySpace.PSUM) as pss,
        ):
            # ---- DMA loads, split across the two HWDGE queues (sync,
            # scalar), ordered by first use.  Expert e0 loads on sync,
            # e1 on scalar so the two CT pipelines stream independently.
            # Small tensors go first (cheap issues, needed early).
            w3rs = []
            for e in range(EPC):
                t = sb.tile([1, H], fp32, tag=f"w3r_{e}", name=f"w3r_{e}")
                eng = nc.sync if e == 0 else nc.scalar
                eng.dma_start(t[:], w3_d[e : e + 1, :])
                w3rs.append(t)
            optf = sb.tile([1, B], fp32, tag="optf", name="optf")
            nc.scalar.dma_start(optf[:], optf_d[:])

            # ---- GpSimd: broadcast W3[e] rows across partitions (early,
            # off the DMA queues and off the critical engines)
            w3bs = []
            for e in range(EPC):
                t = sb.tile([P, H], fp32, tag=f"w3b_{e}", name=f"w3b_{e}")
                nc.gpsimd.partition_broadcast(t[:], w3rs[e][:])
                w3bs.append(t)

            # w1t[e]: [128, 512] tile; column block c holds W1[e].T chunk c
            # (partition = h within chunk).  DRAM view [H, I] -> [c, p, i].
            w1t_view = w1t_d.rearrange("e (c p) i -> e p c i", p=P)
            w1ts = []
            for e in range(EPC):
                t = sb.tile([P, NH * I], fp32, tag=f"w1t_{e}", name=f"w1t_{e}")
                eng = nc.sync if e == 0 else nc.scalar
                eng.dma_start(t[:], w1t_view[e])
                w1ts.append(t)

            # w2[e]: [128, 2048] tile; column block h holds W2[e] rows
            # h*128..h*128+127 (partition = h within chunk).  Chunked into
            # 256 KB DMAs so the CT matmuls start on the first chunk.
            w2_view = w2_d.rearrange("e (h p) k -> e p h k", p=P)
            w2s = []
            for e in range(EPC):
                t = sb.tile([P, NH * H], fp32, tag=f"w2_{e}", name=f"w2_{e}")
                eng = nc.sync if e == 0 else nc.scalar
                for h in range(NH):
                    eng.dma_start(t[:, h * H : (h + 1) * H], w2_view[e][:, h, :])
                w2s.append(t)

            # state^T as one contiguous [128, 1024] load (host-transposed);
            # needed last (by the score matmuls).
            ST = sb.tile([I, B], fp32, tag="ST", name="ST")
            nc.sync.dma_start(ST[:], statet_d[:])

            # ---- selection masks (early; only needs optf).  The host
            # passes optf pre-shifted by the core's expert base, so the
            # comparison constants are the same 0.0/1.0 on every core.
            eqs = {}
            for e in range(EPC):
                eq = sb.tile([1, B], fp32, tag=f"eq_{e}", name=f"eq_{e}")
                nc.vector.tensor_single_scalar(
                    eq[:], optf[:], float(e), mybir.AluOpType.is_equal
                )
                eqs[e] = eq

            # ---- per expert: CT[e] = W1[e] @ W2[e]; V[:,e] = CT[e] @ W3[e];
            # sT[e] = V[:,e]^T @ ST; outp[e] = sT[e] * (option == e).
            # The full chain for e0 runs while e1's CT matmuls stream.
            V = sb.tile([P, EPC], fp32, tag="V", name="V")
            for e in range(EPC):
                ct = psc.tile([P, H], fp32, tag="ct", name="ct")
                for h in range(NH):
                    nc.tensor.matmul(
                        ct[:],
                        w1ts[e][:, h * I : (h + 1) * I],
                        w2s[e][:, h * H : (h + 1) * H],
                        start=(h == 0),
                        stop=(h == NH - 1),
                    )
                junk = sb2.tile([P, H], fp32, tag="junk", name="junk")
                nc.vector.tensor_mul(junk[:], ct[:], w3bs[e][:])
                nc.vector.tensor_reduce(
                    V[:, e : e + 1],
                    junk[:],
                    axis=mybir.AxisListType.X,
                    op=mybir.AluOpType.add,
                )
                outp = sb.tile([1, B], fp32, tag=f"outp_{e}", name=f"outp_{e}")
                for half in range(B // NS):
                    stp = pss.tile([1, NS], fp32, tag="stp", name="stp")
                    nc.tensor.matmul(
                        stp[:],
                        V[:, e : e + 1],
                        ST[:, half * NS : (half + 1) * NS],
                        start=True,
                        stop=True,
                    )
                    nc.vector.tensor_mul(
                        outp[:, half * NS : (half + 1) * NS],
                        stp[:],
                        eqs[e][:, half * NS : (half + 1) * NS],
                    )
                nc.sync.dma_start(out_d[e : e + 1, :], outp[:])

    nc.compile()
    return nc


def _get_nc():
    if "nc" not in _CACHE:
        _CACHE["nc"] = _build_nc()
    return _CACHE["nc"]


def kernel(state, action, W1, W2, W3, option):
    global _LAST_RESULTS
    from concourse import bass_utils

    nc = _get_nc()

    state = np.asarray(state, dtype=np.float32)
    statet = np.ascontiguousarray(state.T)
    W1 = np.asarray(W1, dtype=np.float32)
    w1t = np.ascontiguousarray(np.transpose(W1, (0, 2, 1)))  # [O, H, I]
    W2 = np.asarray(W2, dtype=np.float32)
    W3 = np.asarray(W3, dtype=np.float32)
    opt = np.asarray(option).astype(np.float32).reshape(1, B)

    in_maps = []
    for c in range(NCORES):
        e0 = EPC * c
        in_maps.append(
            {
                "statet": statet,
                "w1t": np.ascontiguousarray(w1t[e0 : e0 + EPC]),
                "w2": np.ascontiguousarray(W2[e0 : e0 + EPC]),
                "w3": np.ascontiguousarray(W3[e0 : e0 + EPC, :, 0]),
                "optf": opt - np.float32(e0),
            }
        )

    res = bass_utils.run_bass_kernel_spmd(
        nc, in_maps, core_ids=list(range(NCORES)), trace=_TRACE
    )
    _LAST_RESULTS = res

    out = np.zeros((B,), np.float32)
    for c in range(NCORES):
        out += res.results[c]["outp"].sum(axis=0)
    return out.reshape(B, 1)


# revision 15
# speedup vs baseline: 1.1289x; 1.0165x over previous
"""Bass/Trainium2 kernel for nn_DifferentSoftQNetwork.

Math: the reference is three chained per-sample-expert matmuls with NO
nonlinearity:
    out[b] = state[b] @ W1[o_b] @ W2[o_b] @ W3[o_b],   o_b = option[b]
Because it is linear, collapse the weight chain per expert:
    v[e] = W1[e] @ W2[e] @ W3[e]  in R^128
    out[b] = dot(state[b], v[o_b])
This turns a 672-MFLOP batched matmul into a memory-bound stream of the
weights (~21 MB).

Sharding: experts are sharded across the 8 cores (2 experts per core);
state/option are replicated.  Each core computes scores s[e, b] for its two
experts and masks them by (option == e); the host sums the 8 partial
[2, 1024] outputs (each b matches exactly one (core, expert) pair).

Input marshalling on the host passes state and W1 pre-transposed (stateT
[I, B], W1T [E, H, I]) so every device-side matmul consumes its natural
layout directly (PE matmul computes lhsT.T @ rhs and needs the contraction
dim on partitions for both operands).

Per-core device program (experts e0, e1):
  CT[e]     = W1T[e].T @ W2[e] = W1[e] @ W2[e]    ([128i, 512k] in PSUM)
  w3b[e]    = partition_broadcast(W3[e])          (GpSimd)
  V[:, e]   = reduce_k(CT[e] * w3b[e])            (DVE mul + reduce)
  sT        = V^T @ ST                            ([2, 1024] scores)
  out[e, b] = sT[e, b] * (option[b] == e)         (DVE is_equal + mul)

DMA issue is split across the two HWDGE queues (sync + scalar) with few,
large transfers; the host-side gather sums the partial outputs.
"""

import numpy as np

B, I, H, O = 1024, 128, 512, 16
NCORES = 8
EPC = O // NCORES  # experts per core = 2

_CACHE = {}

# Internal knobs for the local test harness (the grading harness never
# touches these): when _TRACE is set, the next kernel() call runs with
# NTFF profiling and stores the BassKernelResults in _LAST_RESULTS.
_TRACE = False
_LAST_RESULTS = None

# dummy PE matmuls at kernel start to lift the HAM clock gate
N_WARMUP = 12


def _build_nc():
    import concourse.bacc as bacc
    import concourse.bass as bass
    import concourse.mybir as mybir
    import concourse.tile as tile

    fp32 = mybir.dt.float32
    P = 128

    nc = bacc.Bacc("TRN2", target_bir_lowering=False, debug=False)

    statet_d = nc.dram_tensor("statet", [I, B], fp32, kind="ExternalInput")
    w1t_d = nc.dram_tensor("w1t", [EPC, H, I], fp32, kind="ExternalInput")
    w2_d = nc.dram_tensor("w2", [EPC, H, H], fp32, kind="ExternalInput")
    w3_d = nc.dram_tensor("w3", [EPC, H], fp32, kind="ExternalInput")
    optf_d = nc.dram_tensor("optf", [1, B], fp32, kind="ExternalInput")
    ce_d = nc.dram_tensor("ce01", [EPC, 1], fp32, kind="ExternalInput")
    out_d = nc.dram_tensor("outp", [EPC, B], fp32, kind="ExternalOutput")

    NH = H // P  # 4 chunks of the hidden dim
    NS = 512  # max moving free dim per matmul

    with tile.TileContext(nc) as tc:
        with (
            tc.tile_pool(name="sb", bufs=1) as sb,
            tc.tile_pool(name="sb2", bufs=2) as sb2,
            tc.tile_pool(name="psc", bufs=2, space=bass.MemorySpace.PSUM) as psc,
            tc.tile_pool(name="pss", bufs=2, space=bass.MemorySpace.PSUM) as pss,
        ):
            # ---- DMA loads, split symmetrically across the two HWDGE
            # queues (sync drives expert 0, scalar drives expert 1), in
            # first-use order: w3 row, W1T, W2 chunks, half of state^T.
            engs = [nc.sync, nc.scalar]
            w1t_view = w1t_d.rearrange("e (c p) i -> e p c i", p=P)
            w2_view = w2_d.rearrange("e (h p) k -> e p h k", p=P)
            w3rs, w1ts, w2s = [], [], []
            ST = sb.tile([I, B], fp32, tag="ST", name="ST")
            optf = sb.tile([EPC, B], fp32, tag="optf", name="optf")
            ce01 = sb.tile([EPC, 1], fp32, tag="ce01", name="ce01")
            for e in range(EPC):
                eng = engs[e]
                t = sb.tile([1, H], fp32, tag=f"w3r_{e}", name=f"w3r_{e}")
                eng.dma_start(t[:], w3_d[e : e + 1, :])
                w3rs.append(t)
                if e == 1:
                    eng.dma_start(optf[:], optf_d[0:1, :].to_broadcast([EPC, B]))
                else:
                    eng.dma_start(ce01[:], ce_d[:])
                t = sb.tile([P, NH * I], fp32, tag=f"w1t_{e}", name=f"w1t_{e}")
                eng.dma_start(t[:], w1t_view[e])
                w1ts.append(t)
                t = sb.tile([P, NH * H], fp32, tag=f"w2_{e}", name=f"w2_{e}")
                for h in range(NH):
                    eng.dma_start(t[:, h * H : (h + 1) * H], w2_view[e][:, h, :])
                w2s.append(t)
                half = B // EPC
                eng.dma_start(
                    ST[:, e * half : (e + 1) * half],
                    statet_d[:, e * half : (e + 1) * half],
                )

            # ---- GpSimd: broadcast W3[e] rows across partitions (early,
            # off the DMA queues and off the critical engines)
            w3bs = []
            for e in range(EPC):
                t = sb.tile([P, H], fp32, tag=f"w3b_{e}", name=f"w3b_{e}")
                nc.gpsimd.partition_broadcast(t[:], w3rs[e][:])
                w3bs.append(t)

            # ---- PE warm-up: dummy matmuls so the HAM clock gate reaches
            # 2.4 GHz before the real contraction starts (PE would
            # otherwise run the whole kernel at 1.2 GHz).
            wz = sb.tile([P, 256], fp32, tag="wz", name="wz")
            nc.vector.memset(wz[:], 0.0)
            wp = psc.tile([P, 256], fp32, tag="wp", name="wp", bufs=1)
            for _ in range(N_WARMUP):
                nc.tensor.matmul(wp[:], wz[:, :P], wz[:], start=True, stop=True)

            # ---- selection masks (early; only needs optf).  The host
            # passes optf pre-shifted by the core's expert base, so row e
            # compares against the constant e (same program on all cores).
            eq = sb.tile([EPC, B], fp32, tag="eq", name="eq")
            nc.vector.tensor_scalar(
                eq[:], optf[:], ce01[:], None, op0=mybir.AluOpType.is_equal
            )

            # ---- per expert: CT[e] = W1[e] @ W2[e]; V[:,e] = CT[e] @ W3[e]
            V = sb.tile([P, EPC], fp32, tag="V", name="V")
            for e in range(EPC):
                ct = psc.tile([P, H], fp32, tag="ct", name="ct")
                for h in range(NH):
                    nc.tensor.matmul(
                        ct[:],
                        w1ts[e][:, h * I : (h + 1) * I],
                        w2s[e][:, h * H : (h + 1) * H],
                        start=(h == 0),
                        stop=(h == NH - 1),
                    )
                junk = sb2.tile([P, H], fp32, tag="junk", name="junk")
                nc.vector.tensor_mul(junk[:], ct[:], w3bs[e][:])
                nc.vector.tensor_reduce(
                    V[:, e : e + 1],
                    junk[:],
                    axis=mybir.AxisListType.X,
                    op=mybir.AluOpType.add,
                )

            # ---- scores for both experts at once, then masked output
            outp = sb.tile([EPC, B], fp32, tag="outp", name="outp")
            for half in range(B // NS):
                stp = pss.tile([EPC, NS], fp32, tag="stp", name="stp")
                nc.tensor.matmul(
                    stp[:],
                    V[:],
                    ST[:, half * NS : (half + 1) * NS],
                    start=True,
                    stop=True,
                )
                nc.vector.tensor_mul(
                    outp[:, half * NS : (half + 1) * NS],
                    stp[:],
                    eq[:, half * NS : (half + 1) * NS],
                )
            nc.sync.dma_start(out_d[:], outp[:])

    nc.compile()
    return nc


def _get_nc():
    if "nc" not in _CACHE:
        _CACHE["nc"] = _build_nc()
    return _CACHE["nc"]


def kernel(state, action, W1, W2, W3, option):
    global _LAST_RESULTS
    from concourse import bass_utils

    nc = _get_nc()

    state = np.asarray(state, dtype=np.float32)
    statet = np.ascontiguousarray(state.T)
    W1 = np.asarray(W1, dtype=np.float32)
    w1t = np.ascontiguousarray(np.transpose(W1, (0, 2, 1)))  # [O, H, I]
    W2 = np.asarray(W2, dtype=np.float32)
    W3 = np.asarray(W3, dtype=np.float32)
    opt = np.asarray(option).astype(np.float32).reshape(1, B)

    in_maps = []
    for c in range(NCORES):
        e0 = EPC * c
        in_maps.append(
            {
                "statet": statet,
                "w1t": np.ascontiguousarray(w1t[e0 : e0 + EPC]),
                "w2": np.ascontiguousarray(W2[e0 : e0 + EPC]),
                "w3": np.ascontiguousarray(W3[e0 : e0 + EPC, :, 0]),
                "optf": opt - np.float32(e0),
                "ce01": np.arange(EPC, dtype=np.float32).reshape(EPC, 1),
            }
        )

    res = bass_utils.run_bass_kernel_spmd(
        nc, in_maps, core_ids=list(range(NCORES)), trace=_TRACE
    )
    _LAST_RESULTS = res

    out = np.zeros((B,), np.float32)
    for c in range(NCORES):
        out += res.results[c]["outp"].sum(axis=0)
    return out.reshape(B, 1)


# revision 18
# speedup vs baseline: 1.2226x; 1.0830x over previous
"""Bass/Trainium2 kernel for nn_DifferentSoftQNetwork.

Math: the reference is three chained per-sample-expert matmuls with NO
nonlinearity:
    out[b] = state[b] @ W1[o_b] @ W2[o_b] @ W3[o_b],   o_b = option[b]
Because it is linear, collapse the weight chain per expert:
    v[e] = W1[e] @ W2[e] @ W3[e]  in R^128
    out[b] = dot(state[b], v[o_b])
This turns a 672-MFLOP batched matmul into a memory-bound stream of the
weights (~21 MB).

Sharding: experts are sharded across the 8 cores (2 experts per core);
state/option are replicated.  Each core computes scores s[e, b] for its two
experts and masks them by (option == e); the host sums the 8 partial
[2, 1024] outputs (each b matches exactly one (core, expert) pair).

Input marshalling on the host passes state and W1 pre-transposed (stateT
[I, B], W1T [E, H, I]) so every device-side matmul consumes its natural
layout directly (PE matmul computes lhsT.T @ rhs and needs the contraction
dim on partitions for both operands).

Per-core device program (experts e0, e1):
  CT[e]     = W1T[e].T @ W2[e] = W1[e] @ W2[e]    ([128i, 512k] in PSUM)
  w3b[e]    = partition_broadcast(W3[e])          (GpSimd)
  V[:, e]   = reduce_k(CT[e] * w3b[e])            (DVE mul + reduce)
  sT        = V^T @ ST                            ([2, 1024] scores)
  out[e, b] = sT[e, b] * (option[b] == e)         (DVE is_equal + mul)

DMA issue is split across the two HWDGE queues (sync + scalar) with few,
large transfers; the host-side gather sums the partial outputs.
"""

import numpy as np

B, I, H, O = 1024, 128, 512, 16
NCORES = 8
EPC = O // NCORES  # experts per core = 2

_CACHE = {}

# Internal knobs for the local test harness (the grading harness never
# touches these): when _TRACE is set, the next kernel() call runs with
# NTFF profiling and stores the BassKernelResults in _LAST_RESULTS.
_TRACE = False
_LAST_RESULTS = None

# dummy PE matmuls at kernel start to lift the HAM clock gate
N_WARMUP = 5


def _build_nc():
    import concourse.bacc as bacc
    import concourse.bass as bass
    import concourse.mybir as mybir
    import concourse.tile as tile

    fp32 = mybir.dt.float32
    fp32r = mybir.dt.float32r
    P = 128

    nc = bacc.Bacc("TRN2", target_bir_lowering=False, debug=False)

    statet_d = nc.dram_tensor("statet", [I, B], fp32r, kind="ExternalInput")
    w1t_d = nc.dram_tensor("w1t", [EPC, H, I], fp32r, kind="ExternalInput")
    w2_d = nc.dram_tensor("w2", [EPC, H, H], fp32r, kind="ExternalInput")
    w3_d = nc.dram_tensor("w3", [EPC, H], fp32, kind="ExternalInput")
    optf_d = nc.dram_tensor("optf", [1, B], fp32, kind="ExternalInput")
    ce_d = nc.dram_tensor("ce01", [EPC, 1], fp32, kind="ExternalInput")
    out_d = nc.dram_tensor("outp", [EPC, B], fp32, kind="ExternalOutput")

    NH = H // P  # 4 chunks of the hidden dim
    NS = 512  # max moving free dim per matmul

    with tile.TileContext(nc) as tc:
        with (
            tc.tile_pool(name="sb", bufs=1) as sb,
            tc.tile_pool(name="sb2", bufs=2) as sb2,
            tc.tile_pool(name="psc", bufs=2, space=bass.MemorySpace.PSUM) as psc,
            tc.tile_pool(name="pss", bufs=2, space=bass.MemorySpace.PSUM) as pss,
        ):
            # ---- DMA loads, split symmetrically across the two HWDGE
            # queues (sync drives expert 0, scalar drives expert 1), in
            # first-use order: w3 row, W1T, W2 chunks, half of state^T.
            engs = [nc.sync, nc.scalar]
            w1t_view = w1t_d.rearrange("e (c p) i -> e p c i", p=P)
            w2_view = w2_d.rearrange("e (h p) k -> e p h k", p=P)
            w3rs, w1ts, w2s = [], [], []
            ST = sb.tile([I, B], fp32r, tag="ST", name="ST")
            optf = sb.tile([EPC, B], fp32, tag="optf", name="optf")
            ce01 = sb.tile([EPC, 1], fp32, tag="ce01", name="ce01")
            for e in range(EPC):
                eng = engs[e]
                t = sb.tile([1, H], fp32, tag=f"w3r_{e}", name=f"w3r_{e}")
                eng.dma_start(t[:], w3_d[e : e + 1, :])
                w3rs.append(t)
                if e == 1:
                    eng.dma_start(optf[:], optf_d[0:1, :].to_broadcast([EPC, B]))
                else:
                    eng.dma_start(ce01[:], ce_d[:])
                t = sb.tile([P, NH * I], fp32r, tag=f"w1t_{e}", name=f"w1t_{e}")
                eng.dma_start(t[:], w1t_view[e])
                w1ts.append(t)
                t = sb.tile([P, NH * H], fp32r, tag=f"w2_{e}", name=f"w2_{e}")
                for h in range(NH):
                    eng.dma_start(t[:, h * H : (h + 1) * H], w2_view[e][:, h, :])
                w2s.append(t)
                half = B // EPC
                eng.dma_start(
                    ST[:, e * half : (e + 1) * half],
                    statet_d[:, e * half : (e + 1) * half],
                )

            # ---- GpSimd: broadcast W3[e] rows across partitions (early,
            # off the DMA queues and off the critical engines)
            w3bs = []
            for e in range(EPC):
                t = sb.tile([P, H], fp32, tag=f"w3b_{e}", name=f"w3b_{e}")
                nc.gpsimd.partition_broadcast(t[:], w3rs[e][:])
                w3bs.append(t)

            # ---- PE warm-up: dummy matmuls so the HAM clock gate reaches
            # 2.4 GHz before the real contraction starts (PE would
            # otherwise run the whole kernel at 1.2 GHz).
            wz = sb.tile([P, 256], fp32, tag="wz", name="wz")
            nc.vector.memset(wz[:], 0.0)
            wp = psc.tile([P, 256], fp32, tag="wp", name="wp", bufs=1)
            for _ in range(N_WARMUP):
                nc.tensor.matmul(wp[:], wz[:, :P], wz[:], start=True, stop=True)

            # ---- selection masks (early; only needs optf).  The host
            # passes optf pre-shifted by the core's expert base, so row e
            # compares against the constant e (same program on all cores).
            eq = sb.tile([EPC, B], fp32, tag="eq", name="eq")
            nc.vector.tensor_scalar(
                eq[:], optf[:], ce01[:], None, op0=mybir.AluOpType.is_equal
            )

            # ---- per expert: CT[e] = W1[e] @ W2[e]; V[:,e] = CT[e] @ W3[e]
            V = sb.tile([P, EPC], fp32r, tag="V", name="V")
            for e in range(EPC):
                ct = psc.tile([P, H], fp32, tag="ct", name="ct")
                for h in range(NH):
                    nc.tensor.matmul(
                        ct[:],
                        w1ts[e][:, h * I : (h + 1) * I],
                        w2s[e][:, h * H : (h + 1) * H],
                        start=(h == 0),
                        stop=(h == NH - 1),
                    )
                junk = sb2.tile([P, H], fp32, tag="junk", name="junk")
                nc.vector.tensor_mul(junk[:], ct[:], w3bs[e][:])
                with nc.allow_low_precision(reason="fp32r V for fast PE"):
                    nc.vector.tensor_reduce(
                        V[:, e : e + 1],
                        junk[:],
                        axis=mybir.AxisListType.X,
                        op=mybir.AluOpType.add,
                    )

            # ---- scores for both experts at once, then masked output
            outp = sb.tile([EPC, B], fp32, tag="outp", name="outp")
            for half in range(B // NS):
                stp = pss.tile([EPC, NS], fp32, tag="stp", name="stp")
                nc.tensor.matmul(
                    stp[:],
                    V[:],
                    ST[:, half * NS : (half + 1) * NS],
                    start=True,
                    stop=True,
                )
                nc.vector.tensor_mul(
                    outp[:, half * NS : (half + 1) * NS],
                    stp[:],
                    eq[:, half * NS : (half + 1) * NS],
                )
            nc.sync.dma_start(out_d[:], outp[:])

    nc.compile()
    return nc


def _get_nc():
    if "nc" not in _CACHE:
        _CACHE["nc"] = _build_nc()
    return _CACHE["nc"]


def kernel(state, action, W1, W2, W3, option):
    global _LAST_RESULTS
    from concourse import bass_utils

    nc = _get_nc()

    state = np.asarray(state, dtype=np.float32)
    statet = np.ascontiguousarray(state.T)
    W1 = np.asarray(W1, dtype=np.float32)
    w1t = np.ascontiguousarray(np.transpose(W1, (0, 2, 1)))  # [O, H, I]
    W2 = np.asarray(W2, dtype=np.float32)
    W3 = np.asarray(W3, dtype=np.float32)
    opt = np.asarray(option).astype(np.float32).reshape(1, B)

    in_maps = []
    for c in range(NCORES):
        e0 = EPC * c
        in_maps.append(
            {
                "statet": statet,
                "w1t": np.ascontiguousarray(w1t[e0 : e0 + EPC]),
                "w2": np.ascontiguousarray(W2[e0 : e0 + EPC]),
                "w3": np.ascontiguousarray(W3[e0 : e0 + EPC, :, 0]),
                "optf": opt - np.float32(e0),
                "ce01": np.arange(EPC, dtype=np.float32).reshape(EPC, 1),
            }
        )

    res = bass_utils.run_bass_kernel_spmd(
        nc, in_maps, core_ids=list(range(NCORES)), trace=_TRACE
    )
    _LAST_RESULTS = res

    out = np.zeros((B,), np.float32)
    for c in range(NCORES):
        out += res.results[c]["outp"].sum(axis=0)
    return out.reshape(B, 1)


# revision 21
# speedup vs baseline: 1.3369x; 1.0935x over previous
"""Bass/Trainium2 kernel for nn_DifferentSoftQNetwork.

Math: the reference is three chained per-sample-expert matmuls with NO
nonlinearity:
    out[b] = state[b] @ W1[o_b] @ W2[o_b] @ W3[o_b],   o_b = option[b]
Because it is linear, collapse the weight chain per expert:
    v[e] = W1[e] @ W2[e] @ W3[e]  in R^128
    out[b] = dot(state[b], v[o_b])
This turns a 672-MFLOP batched matmul into a memory-bound stream of the
weights (~21 MB).

Sharding: experts are sharded across the 8 cores (2 experts per core);
state/option are replicated.  Each core computes scores s[e, b] for its two
experts and masks them by (option == e); the host sums the 8 partial
[2, 1024] outputs (each b matches exactly one (core, expert) pair).

Input marshalling on the host passes state and W1 pre-transposed (stateT
[I, B], W1T [E, H, I]) so every device-side matmul consumes its natural
layout directly (PE matmul computes lhsT.T @ rhs and needs the contraction
dim on partitions for both operands).

Per-core device program (experts e0, e1):
  CT[e]     = W1T[e].T @ W2[e] = W1[e] @ W2[e]    ([128i, 512k] in PSUM)
  w3b[e]    = partition_broadcast(W3[e])          (GpSimd)
  V[:, e]   = reduce_k(CT[e] * w3b[e])            (DVE mul + reduce)
  sT        = V^T @ ST                            ([2, 1024] scores)
  out[e, b] = sT[e, b] * (option[b] == e)         (DVE is_equal + mul)

DMA issue is split across the two HWDGE queues (sync + scalar) with few,
large transfers; the host-side gather sums the partial outputs.
"""

import numpy as np

B, I, H, O = 1024, 128, 512, 16
NCORES = 8
EPC = O // NCORES  # experts per core = 2

_CACHE = {}

# Internal knobs for the local test harness (the grading harness never
# touches these): when _TRACE is set, the next kernel() call runs with
# NTFF profiling and stores the BassKernelResults in _LAST_RESULTS.
_TRACE = False
_LAST_RESULTS = None

# dummy PE matmuls at kernel start to lift the HAM clock gate
N_WARMUP = 4


def _build_nc():
    import concourse.bacc as bacc
    import concourse.bass as bass
    import concourse.mybir as mybir
    import concourse.tile as tile

    fp32 = mybir.dt.float32
    fp32r = mybir.dt.float32r
    P = 128

    nc = bacc.Bacc("TRN2", target_bir_lowering=False, debug=False)

    statet_d = nc.dram_tensor("statet", [I, B], fp32r, kind="ExternalInput")
    w1t_d = nc.dram_tensor("w1t", [EPC, H, I], fp32r, kind="ExternalInput")
    w2_d = nc.dram_tensor("w2", [EPC, H, H], fp32r, kind="ExternalInput")
    w3_d = nc.dram_tensor("w3", [EPC, H], fp32, kind="ExternalInput")
    optf_d = nc.dram_tensor("optf", [1, B], fp32, kind="ExternalInput")
    ce_d = nc.dram_tensor("ce01", [EPC, 1], fp32, kind="ExternalInput")
    out_d = nc.dram_tensor("outp", [EPC, B], fp32, kind="ExternalOutput")

    NH = H // P  # 4 chunks of the hidden dim
    NS = 512  # max moving free dim per matmul

    with tile.TileContext(nc) as tc:
        with (
            tc.tile_pool(name="sb", bufs=1) as sb,
            tc.tile_pool(name="sb2", bufs=2) as sb2,
            tc.tile_pool(name="psc", bufs=2, space=bass.MemorySpace.PSUM) as psc,
            tc.tile_pool(name="pss", bufs=2, space=bass.MemorySpace.PSUM) as pss,
        ):
            # ---- DMA loads, split symmetrically across the two HWDGE
            # queues (sync drives expert 0, scalar drives expert 1), in
            # first-use order: W1T, W2 chunks (last one split so the tail
            # matmul waits on 128 KB, not 256 KB), state^T half, then the
            # small late-use tensors (w3 row mid-queue, optf last).
            engs = [nc.sync, nc.scalar]
            w1t_view = w1t_d.rearrange("e (c p) i -> e p c i", p=P)
            w2_view = w2_d.rearrange("e (h p) k -> e p h k", p=P)
            w3rs, w1ts, w2s = [], [], []
            ST = sb.tile([I, B], fp32r, tag="ST", name="ST")
            optf = sb.tile([EPC, B], fp32, tag="optf", name="optf")
            ce01 = sb.tile([EPC, 1], fp32, tag="ce01", name="ce01")
            for e in range(EPC):
                eng = engs[e]
                t = sb.tile([P, NH * I], fp32r, tag=f"w1t_{e}", name=f"w1t_{e}")
                eng.dma_start(t[:], w1t_view[e])
                w1ts.append(t)
                w2t = sb.tile([P, NH * H], fp32r, tag=f"w2_{e}", name=f"w2_{e}")
                for h in range(NH):
                    eng.dma_start(
                        w2t[:, h * H : (h + 1) * H], w2_view[e][:, h, :]
                    )
                    if h == 0:
                        w3t = sb.tile([1, H], fp32, tag=f"w3r_{e}", name=f"w3r_{e}")
                        eng.dma_start(w3t[:], w3_d[e : e + 1, :])
                        w3rs.append(w3t)
                w2s.append(w2t)
                half = B // EPC
                eng.dma_start(
                    ST[:, e * half : (e + 1) * half],
                    statet_d[:, e * half : (e + 1) * half],
                )
            nc.scalar.dma_start(optf[:], optf_d[0:1, :].to_broadcast([EPC, B]))
            nc.scalar.dma_start(ce01[:], ce_d[:])

            # ---- GpSimd: broadcast W3[e] rows across partitions (early,
            # off the DMA queues and off the critical engines)
            w3bs = []
            for e in range(EPC):
                t = sb.tile([P, H], fp32, tag=f"w3b_{e}", name=f"w3b_{e}")
                nc.gpsimd.partition_broadcast(t[:], w3rs[e][:])
                w3bs.append(t)

            # ---- PE warm-up: dummy matmuls so the HAM clock gate reaches
            # 2.4 GHz before the real contraction starts (PE would
            # otherwise run the whole kernel at 1.2 GHz).
            wz = sb.tile([P, 256], fp32, tag="wz", name="wz")
            nc.vector.memset(wz[:], 0.0)
            wp = psc.tile([P, 256], fp32, tag="wp", name="wp", bufs=1)
            for _ in range(N_WARMUP):
                nc.tensor.matmul(wp[:], wz[:, :P], wz[:], start=True, stop=True)

            # ---- selection masks (early; only needs optf).  The host
            # passes optf pre-shifted by the core's expert base, so row e
            # compares against the constant e (same program on all cores).
            eq = sb.tile([EPC, B], fp32, tag="eq", name="eq")
            nc.vector.tensor_scalar(
                eq[:], optf[:], ce01[:], None, op0=mybir.AluOpType.is_equal
            )

            # ---- per expert: CT[e] = W1[e] @ W2[e]; V[:,e] = CT[e] @ W3[e]
            V = sb.tile([P, EPC], fp32r, tag="V", name="V")
            for e in range(EPC):
                ct = psc.tile([P, H], fp32, tag="ct", name="ct")
                for h in range(NH):
                    nc.tensor.matmul(
                        ct[:],
                        w1ts[e][:, h * I : (h + 1) * I],
                        w2s[e][:, h * H : (h + 1) * H],
                        start=(h == 0),
                        stop=(h == NH - 1),
                    )
                junk = sb2.tile([P, H], fp32, tag="junk", name="junk")
                with nc.allow_low_precision(reason="fp32r V for fast PE"):
                    nc.vector.scalar_tensor_tensor(
                        junk[:],
                        ct[:],
                        1.0,
                        w3bs[e][:],
                        op0=mybir.AluOpType.mult,
                        op1=mybir.AluOpType.mult,
                        accum_out=V[:, e : e + 1],
                    )

            # ---- scores for both experts at once, then masked output
            outp = sb.tile([EPC, B], fp32, tag="outp", name="outp")
            for half in range(B // NS):
                stp = pss.tile([EPC, NS], fp32, tag="stp", name="stp")
                nc.tensor.matmul(
                    stp[:],
                    V[:],
                    ST[:, half * NS : (half + 1) * NS],
                    start=True,
                    stop=True,
                )
                nc.vector.tensor_mul(
                    outp[:, half * NS : (half + 1) * NS],
                    stp[:],
                    eq[:, half * NS : (half + 1) * NS],
                )
            nc.sync.dma_start(out_d[:], outp[:])

    nc.compile()
    return nc


def _get_nc():
    if "nc" not in _CACHE:
        _CACHE["nc"] = _build_nc()
    return _CACHE["nc"]


def kernel(state, action, W1, W2, W3, option):
    global _LAST_RESULTS
    from concourse import bass_utils

    nc = _get_nc()

    state = np.asarray(state, dtype=np.float32)
    statet = np.ascontiguousarray(state.T)
    W1 = np.asarray(W1, dtype=np.float32)
    w1t = np.ascontiguousarray(np.transpose(W1, (0, 2, 1)))  # [O, H, I]
    W2 = np.asarray(W2, dtype=np.float32)
    W3 = np.asarray(W3, dtype=np.float32)
    opt = np.asarray(option).astype(np.float32).reshape(1, B)

    in_maps = []
    for c in range(NCORES):
        e0 = EPC * c
        in_maps.append(
            {
                "statet": statet,
                "w1t": np.ascontiguousarray(w1t[e0 : e0 + EPC]),
                "w2": np.ascontiguousarray(W2[e0 : e0 + EPC]),
                "w3": np.ascontiguousarray(W3[e0 : e0 + EPC, :, 0]),
                "optf": opt - np.float32(e0),
                "ce01": np.arange(EPC, dtype=np.float32).reshape(EPC, 1),
            }
        )

    res = bass_utils.run_bass_kernel_spmd(
        nc, in_maps, core_ids=list(range(NCORES)), trace=_TRACE
    )
    _LAST_RESULTS = res

    out = np.zeros((B,), np.float32)
    for c in range(NCORES):
        out += res.results[c]["outp"].sum(axis=0)
    return out.reshape(B, 1)
